# revision 5
# baseline (speedup 1.0000x reference)
"""GATv2 encoder (2x relational GATv2Conv + linear head + layernorm) on 8 trn2 cores.

Sharding: core k owns dst nodes [4096k, 4096(k+1)); edges partitioned by dst.
Per layer, two passes over the core's edges with all data-dependent structure
(indices, window bases, dst-local ids, scatter positions) carried as tensor
data so the instruction stream is identical across cores (single SPMD program):

  P1 (relation-sorted, common segment sizes): ea = gelu(x[src]@Rl[et] +
     x[dst]@Rr[et]) computed feature-major via transposed DMA gathers and
     per-relation matmuls, PE-transposed to edge-major and indirect-scattered
     into ea_dram at each edge's dst-order slot.
  P2 (dst-sorted, fixed 2048-edge windows of <=128 nodes): m = We@ea + XL[src]
     + XL[dst] (XL = x@Wl + bl precomputed node-major); alpha = attblk @
     leaky(m); exp (segment-max skipped: |alpha| <~ 1.1); msg = XL[src] *
     exp(alpha)[head]; windowed segment softmax-sum via one-hot S-matrix
     matmuls accumulating [128 nodes x (128 msg + 4 expsum)] in PSUM;
     divide, +bias (+gelu after layer 0); indirect-scatter rows to the local
     node slab. AllGather slabs -> full x for the next layer.
Head: every core computes all 8192 rows (x2 @ out_w + b, layernorm); host
takes core 0's output.
"""
import sys
sys.path.insert(0, "/opt/trn_rl_repo")
import numpy as np
import ml_dtypes

import concourse.bass as bass
import concourse.bacc as bacc
import concourse.mybir as mybir
import concourse.tile as tile
from concourse.bass_utils import run_bass_kernel_spmd

F32, BF16, I16, I32 = mybir.dt.float32, mybir.dt.bfloat16, mybir.dt.int16, mybir.dt.int32
AF = mybir.ActivationFunctionType
ALU = mybir.AluOpType

N, E, D, H, C, R, BS = 32768, 524288, 128, 4, 32, 5, 8192
NEG = 0.2
NC = 8
NPC = N // NC
CH = 512
GW = 16
WEDGE = GW * 128
P = 128

_bf = ml_dtypes.bfloat16


def _wrap_idx(idx):
    m = idx.reshape(len(idx) // 16, 16).T.astype(np.int16)
    return np.tile(m, (8, 1))


def _col_layout(a, dt):
    return np.ascontiguousarray(a.reshape(-1, P).T).astype(dt)


def _bcast_row(v):
    return np.ascontiguousarray(
        np.broadcast_to(np.asarray(v, np.float32)[None, :], (P, D)))


def _prep_core(k, src, dst, et, S_r, W):
    sel = np.where((dst >= k * NPC) & (dst < (k + 1) * NPC))[0]
    d_loc = dst[sel] - k * NPC
    order = np.argsort(d_loc, kind="stable")
    sel, d_loc = sel[order], d_loc[order]
    ne = len(sel)

    deg = np.bincount(d_loc, minlength=NPC)
    csum = np.concatenate([[0], np.cumsum(deg)])
    wins = []
    n0 = 0
    while n0 < NPC:
        n1 = min(n0 + P, NPC)
        while csum[n1] - csum[n0] > WEDGE:
            n1 -= 1
        wins.append((n0, n1))
        n0 = n1
    assert len(wins) <= W, (len(wins), W)
    L2 = W * WEDGE
    p2_src = np.zeros(L2, np.int64)
    p2_dst = np.zeros(L2, np.int64)
    p2_dstloc = np.full(L2, 255, np.int64)
    p2_pos_of_edge = np.zeros(ne, np.int64)
    rowidx = np.zeros((W, P), np.int32)
    for w in range(W):
        if w < len(wins):
            n0, n1 = wins[w]
            lo, hi = int(csum[n0]), int(csum[n1])
            cnt = hi - lo
            base = w * WEDGE
            p2_src[base:base + cnt] = src[sel[lo:hi]]
            p2_dst[base:base + cnt] = dst[sel[lo:hi]]
            p2_dstloc[base:base + cnt] = d_loc[lo:hi] - n0
            p2_pos_of_edge[lo:hi] = base + np.arange(cnt)
            span = n1 - n0
            rowidx[w, :span] = n0 + np.arange(span)
            rowidx[w, span:] = NPC + np.arange(P - span)
        else:
            rowidx[w] = NPC + np.arange(P)

    et_k = et[sel]
    L1 = int(S_r.sum())
    p1_src = np.zeros(L1, np.int64)
    p1_dst = np.zeros(L1, np.int64)
    eapos = np.full(L1, L2, np.int64)
    off = 0
    for r in range(R):
        ids = np.where(et_k == r)[0]
        cnt = len(ids)
        p1_src[off:off + cnt] = src[sel[ids]]
        p1_dst[off:off + cnt] = dst[sel[ids]]
        eapos[off:off + cnt] = p2_pos_of_edge[ids]
        pad = int(S_r[r]) - cnt
        eapos[off + cnt:off + int(S_r[r])] = L2 + (np.arange(pad) % P)
        off += int(S_r[r])

    return {
        "p1_src": _wrap_idx(p1_src), "p1_dst": _wrap_idx(p1_dst),
        "eapos": _col_layout(eapos, np.int32),
        "p2_src": _wrap_idx(p2_src), "p2_dst": _wrap_idx(p2_dst),
        "dstloc": _col_layout(p2_dstloc, np.float32).astype(_bf),
        "rowidx": np.ascontiguousarray(rowidx.T).astype(np.int32),
    }


def build_nc(S_r, W, reps=1):
    NCH1 = int(S_r.sum()) // CH
    NCH2 = W * WEDGE // CH
    L2 = W * WEDGE
    rel_of_chunk = np.repeat(np.arange(R), S_r // CH)

    nc = bacc.Bacc("TRN2", target_bir_lowering=False, debug=False, num_devices=NC)
    din = lambda name, shape, dt: nc.dram_tensor(name, shape, dt, kind="ExternalInput")

    embs_bf = din("embs_bf", [N, D], BF16)
    wRl = [[din(f"wRl_{l}_{r}", [D, D], BF16) for r in range(R)] for l in range(2)]
    wRr = [[din(f"wRr_{l}_{r}", [D, D], BF16) for r in range(R)] for l in range(2)]
    wWe = [din(f"wWe_{l}", [D, D], BF16) for l in range(2)]
    wWl = [din(f"wWl_{l}", [D, D], BF16) for l in range(2)]
    attb = [din(f"attblk_{l}", [D, H], BF16) for l in range(2)]
    e4d = din("e4", [H, D], BF16)
    bias_bc = [din(f"bias_bc_{l}", [P, D], F32) for l in range(2)]
    blw_bc = [din(f"blw_bc_{l}", [P, D], F32) for l in range(2)]
    outw_bf = din("outw_bf", [D, D], BF16)
    outb_bc = din("outb_bc", [P, D], F32)
    lng_bc = din("lng_bc", [P, D], F32)
    lnb_bc = din("lnb_bc", [P, D], F32)
    p1_src_d = din("p1_src", [P, NCH1 * 32], I16)
    p1_dst_d = din("p1_dst", [P, NCH1 * 32], I16)
    eapos_d = din("eapos", [P, NCH1 * 4], I32)
    p2_src_d = din("p2_src", [P, NCH2 * 32], I16)
    p2_dst_d = din("p2_dst", [P, NCH2 * 32], I16)
    dstloc_d = din("dstloc", [P, NCH2 * 4], BF16)
    rowidx_d = din("rowidx", [P, W], I32)

    y_out = nc.dram_tensor("y_out", [BS, D], F32, kind="ExternalOutput")

    ea_dram = nc.dram_tensor("ea_dram", [L2 + P, D], BF16, kind="Internal")
    xl_dram = nc.dram_tensor("xl_dram", [N, D], BF16, kind="Internal")
    x1_local = nc.dram_tensor("x1_local", [NPC + P, D], BF16, kind="Internal")
    xg = [nc.dram_tensor(f"xg_{l}", [N, D], BF16, kind="Internal",
                         addr_space="Shared") for l in range(2)]

    with tile.TileContext(nc) as tc:
        with tc.tile_pool(name="const", bufs=1) as const, \
             tc.tile_pool(name="sb", bufs=3) as sb, \
             tc.tile_pool(name="sbg", bufs=4) as sbg, \
             tc.tile_pool(name="ps", bufs=1, space="PSUM") as ps, \
             tc.tile_pool(name="psw", bufs=2, space="PSUM") as psw:

            def cload(dram, shape, dt):
                t = const.tile(shape, dt, tag=f"c_{dram.name}")
                nc.sync.dma_start(out=t[:], in_=dram.ap())
                return t

            c_Rl = [[cload(wRl[l][r], [D, D], BF16) for r in range(R)] for l in range(2)]
            c_Rr = [[cload(wRr[l][r], [D, D], BF16) for r in range(R)] for l in range(2)]
            c_We = [cload(wWe[l], [D, D], BF16) for l in range(2)]
            c_Wl = [cload(wWl[l], [D, D], BF16) for l in range(2)]
            c_att = [cload(attb[l], [D, H], BF16) for l in range(2)]
            c_e4 = cload(e4d, [H, D], BF16)
            c_bias = [cload(bias_bc[l], [P, D], F32) for l in range(2)]
            c_blw = [cload(blw_bc[l], [P, D], F32) for l in range(2)]
            c_outw = cload(outw_bf, [D, D], BF16)
            c_outb = cload(outb_bc, [P, D], F32)
            c_lng = cload(lng_bc, [P, D], F32)
            c_lnb = cload(lnb_bc, [P, D], F32)
            c_p1s = cload(p1_src_d, [P, NCH1 * 32], I16)
            c_p1d = cload(p1_dst_d, [P, NCH1 * 32], I16)
            c_eap = cload(eapos_d, [P, NCH1 * 4], I32)
            c_p2s = cload(p2_src_d, [P, NCH2 * 32], I16)
            c_p2d = cload(p2_dst_d, [P, NCH2 * 32], I16)
            c_dl = cload(dstloc_d, [P, NCH2 * 4], BF16)
            c_row = cload(rowidx_d, [P, W], I32)

            from concourse.masks import make_identity
            ident = const.tile([P, P], F32)
            make_identity(nc, ident[:])
            ident_bf = const.tile([P, P], BF16)
            nc.vector.tensor_copy(ident_bf[:], ident[:])
            iota_i = const.tile([P, P], I32)
            nc.gpsimd.iota(iota_i[:], pattern=[[1, P]], base=0, channel_multiplier=0)
            iota_bf = const.tile([P, P], BF16)
            nc.vector.tensor_copy(iota_bf[:], iota_i[:])
            zeros_sb = const.tile([P, D], BF16)
            nc.vector.memset(zeros_sb[:], 0.0)

            eaz = ea_dram.ap().rearrange("(b p) d -> b p d", p=P)
            for b in range((L2 + P) // P):
                nc.sync.dma_start(out=eaz[b], in_=zeros_sb[:])

            def gather(pool, tag, table, idx_tile, c):
                t = pool.tile([P, CH], BF16, tag=tag)
                nc.gpsimd.dma_gather(
                    t[:].rearrange("p (o n) -> p o n", o=1), table.ap(),
                    idx_tile[:, c * 32:(c + 1) * 32], num_idxs=CH,
                    num_idxs_reg=CH, elem_size=D, transpose=True)
                return t

            win_box = [None]

            def body(_iv=None):
                for l in range(2):
                    x_tab = embs_bf if l == 0 else xg[0]

                    # XL = x @ Wl + bl (node-major bf16, all N rows)
                    xlv = xl_dram.ap().rearrange("(b p) d -> b p d", p=P)
                    for b in range(N // CH):
                        xt = sb.tile([P, CH], BF16, tag="xlxt")
                        nc.sync.dma_start_transpose(
                            out=xt[:], in_=x_tab.ap()[b * CH:(b + 1) * CH, :])
                        for j in range(4):
                            blkp = ps.tile([P, CH], F32, tag="bigps")
                            nc.tensor.matmul(blkp[:, :P], lhsT=xt[:, j * P:(j + 1) * P],
                                             rhs=c_Wl[l][:], start=True, stop=True)
                            blk = sb.tile([P, D], BF16, tag="xlblk")
                            nc.vector.tensor_tensor(out=blk[:], in0=blkp[:, :P],
                                                    in1=c_blw[l][:], op=ALU.add)
                            nc.sync.dma_start(out=xlv[b * 4 + j], in_=blk[:])

                    # P1
                    for c in range(NCH1):
                        r = int(rel_of_chunk[c])
                        xs = gather(sbg, "p1xs", x_tab, c_p1s, c)
                        xd = gather(sbg, "p1xd", x_tab, c_p1d, c)
                        t_ps = ps.tile([P, CH], F32, tag="bigps")
                        nc.tensor.matmul(t_ps[:], lhsT=c_Rl[l][r][:], rhs=xs[:],
                                         start=True, stop=False)
                        nc.tensor.matmul(t_ps[:], lhsT=c_Rr[l][r][:], rhs=xd[:],
                                         start=False, stop=True)
                        ea = sb.tile([P, CH], BF16, tag="ea")
                        nc.scalar.activation(ea[:], t_ps[:], AF.Gelu)
                        for g in range(4):
                            eat = ps.tile([P, P], BF16, tag="tps")
                            nc.tensor.transpose(out=eat[:], in_=ea[:, g * P:(g + 1) * P],
                                                identity=ident_bf[:])
                            eam = sb.tile([P, P], BF16, tag="eam")
                            nc.vector.tensor_copy(eam[:], eat[:])
                            nc.gpsimd.indirect_dma_start(
                                out=ea_dram.ap(),
                                out_offset=bass.IndirectOffsetOnAxis(
                                    ap=c_eap[:, c * 4 + g:c * 4 + g + 1], axis=0),
                                in_=eam[:], in_offset=None)

                    # P2
                    for c in range(NCH2):
                        w = c // 4
                        eaf = sb.tile([P, CH], BF16, tag="eaf")
                        nc.sync.dma_start_transpose(
                            out=eaf[:], in_=ea_dram.ap()[c * CH:(c + 1) * CH, :])
                        xls = gather(sbg, "xls", xl_dram, c_p2s, c)
                        xld = gather(sbg, "xld", xl_dram, c_p2d, c)
                        m_ps = ps.tile([P, CH], F32, tag="bigps")
                        nc.tensor.matmul(m_ps[:], lhsT=c_We[l][:], rhs=eaf[:],
                                         start=True, stop=True)
                        mtot = sb.tile([P, CH], F32, tag="mtot")
                        nc.vector.tensor_tensor(out=mtot[:], in0=m_ps[:], in1=xls[:],
                                                op=ALU.add)
                        nc.vector.tensor_tensor(out=mtot[:], in0=mtot[:], in1=xld[:],
                                                op=ALU.add)
                        m02 = sb.tile([P, CH], F32, tag="m02")
                        nc.scalar.mul(m02[:], mtot[:], NEG)
                        lr = sb.tile([P, CH], BF16, tag="lr")
                        nc.vector.tensor_tensor(out=lr[:], in0=mtot[:], in1=m02[:],
                                                op=ALU.max)
                        a_ps = ps.tile([H, CH], F32, tag="aps")
                        nc.tensor.matmul(a_ps[:], lhsT=c_att[l][:], rhs=lr[:],
                                         start=True, stop=True)
                        expa = sb.tile([H, CH], BF16, tag="expa")
                        nc.scalar.activation(expa[:], a_ps[:], AF.Exp)
                        a128_ps = ps.tile([P, CH], F32, tag="a128ps")
                        nc.tensor.matmul(a128_ps[:], lhsT=c_e4[:], rhs=expa[:],
                                         start=True, stop=True)
                        a128 = sb.tile([P, CH], BF16, tag="a128")
                        nc.vector.tensor_copy(a128[:], a128_ps[:])
                        msg = sb.tile([P, CH], BF16, tag="msg")
                        nc.vector.tensor_tensor(out=msg[:], in0=xls[:], in1=a128[:],
                                                op=ALU.mult)
                        if c % 4 == 0:
                            win_t = psw.tile([P, 132], F32, tag="win")
                            win_box[0] = win_t
                        win = win_box[0]
                        for g in range(4):
                            S = sb.tile([P, P], BF16, tag="S")
                            nc.vector.tensor_tensor(
                                out=S[:],
                                in0=c_dl[:, c * 4 + g:c * 4 + g + 1].to_broadcast([P, P]),
                                in1=iota_bf[:], op=ALU.is_equal)
                            rhs_g = sb.tile([P, 132], BF16, tag="rhs_g")
                            msgt = ps.tile([P, P], BF16, tag="tps")
                            nc.tensor.transpose(out=msgt[:], in_=msg[:, g * P:(g + 1) * P],
                                                identity=ident_bf[:])
                            nc.vector.tensor_copy(rhs_g[:, :P], msgt[:])
                            expt = ps.tile([P, H], BF16, tag="expt")
                            nc.tensor.transpose(out=expt[:], in_=expa[:, g * P:(g + 1) * P],
                                                identity=ident_bf[:H, :H])
                            nc.vector.tensor_copy(rhs_g[:, P:P + H], expt[:])
                            nc.tensor.matmul(win[:], lhsT=S[:], rhs=rhs_g[:],
                                             start=(c % 4 == 0 and g == 0),
                                             stop=(c % 4 == 3 and g == 3),
                                             skip_group_check=True)
                        if c % 4 == 3:
                            den = sb.tile([P, H], F32, tag="den")
                            nc.vector.tensor_scalar_add(den[:], win[:, P:P + H], 1e-16)
                            rec = sb.tile([P, H], F32, tag="rec")
                            nc.vector.reciprocal(rec[:], den[:])
                            outn = sb.tile([P, D], F32, tag="outn")
                            nc.vector.tensor_tensor(
                                out=outn[:].rearrange("p (h c) -> p h c", h=H),
                                in0=win[:, :D].rearrange("p (h c) -> p h c", h=H),
                                in1=rec[:, :, None].to_broadcast([P, H, C]),
                                op=ALU.mult)
                            nc.vector.tensor_tensor(out=outn[:], in0=outn[:],
                                                    in1=c_bias[l][:], op=ALU.add)
                            xo = sb.tile([P, D], BF16, tag="xo")
                            if l == 0:
                                nc.scalar.activation(xo[:], outn[:], AF.Gelu)
                            else:
                                nc.vector.tensor_copy(xo[:], outn[:])
                            nc.gpsimd.indirect_dma_start(
                                out=x1_local.ap(),
                                out_offset=bass.IndirectOffsetOnAxis(
                                    ap=c_row[:, w:w + 1], axis=0),
                                in_=xo[:], in_offset=None)

                    nc.gpsimd.collective_compute(
                        "AllGather", ALU.bypass,
                        replica_groups=[list(range(NC))],
                        ins=[x1_local.ap()[0:NPC, :]],
                        outs=[xg[l].ap()],
                    )

                # head: all BS rows on every core
                yv = y_out.ap().rearrange("(b p) d -> b p d", p=P)
                for b in range(BS // P):
                    xt = sb.tile([P, P], BF16, tag="hxt")
                    nc.sync.dma_start_transpose(
                        out=xt[:], in_=xg[1].ap()[b * P:(b + 1) * P, :])
                    y_ps = ps.tile([P, CH], F32, tag="bigps")
                    nc.tensor.matmul(y_ps[:, :P], lhsT=xt[:], rhs=c_outw[:],
                                     start=True, stop=True)
                    y1 = sb.tile([P, D], F32, tag="y1")
                    nc.vector.tensor_tensor(out=y1[:], in0=y_ps[:, :P],
                                            in1=c_outb[:], op=ALU.add)
                    musum = sb.tile([P, 1], F32, tag="musum")
                    nc.vector.reduce_sum(musum[:], y1[:], axis=mybir.AxisListType.X)
                    negmu = sb.tile([P, 1], F32, tag="negmu")
                    nc.scalar.mul(negmu[:], musum[:], -1.0 / D)
                    yc = sb.tile([P, D], F32, tag="yc")
                    nc.scalar.activation(yc[:], y1[:], AF.Identity, bias=negmu[:, :1])
                    sq = sb.tile([P, D], F32, tag="sq")
                    varsum = sb.tile([P, 1], F32, tag="varsum")
                    nc.scalar.activation(sq[:], yc[:], AF.Square,
                                         accum_out=varsum[:, :1])
                    varm = sb.tile([P, 1], F32, tag="varm")
                    nc.scalar.mul(varm[:], varsum[:], 1.0 / D)
                    nc.vector.tensor_scalar_add(varm[:], varm[:], 1e-12)
                    std = sb.tile([P, 1], F32, tag="std")
                    nc.scalar.activation(std[:], varm[:], AF.Sqrt)
                    rstd = sb.tile([P, 1], F32, tag="rstd")
                    nc.vector.reciprocal(rstd[:], std[:])
                    yn = sb.tile([P, D], F32, tag="yn")
                    nc.scalar.mul(yn[:], yc[:], rstd[:, :1])
                    yg2 = sb.tile([P, D], F32, tag="yg2")
                    nc.vector.tensor_tensor(out=yg2[:], in0=yn[:], in1=c_lng[:],
                                            op=ALU.mult)
                    nc.vector.tensor_tensor(out=yg2[:], in0=yg2[:], in1=c_lnb[:],
                                            op=ALU.add)
                    nc.sync.dma_start(out=yv[b], in_=yg2[:])

            for _ in range(reps):
                body()

    nc.compile()
    return nc


def _host_prep(inputs):
    src = np.asarray(inputs["edge_index"])[0].astype(np.int64)
    dst = np.asarray(inputs["edge_index"])[1].astype(np.int64)
    et = np.asarray(inputs["edge_type"]).astype(np.int64)

    # common P1 segment sizes and window count across cores
    seg = np.zeros((NC, R), np.int64)
    wcnt = np.zeros(NC, np.int64)
    for k in range(NC):
        m = (dst >= k * NPC) & (dst < (k + 1) * NPC)
        ek = np.where(m)[0]
        d_loc = dst[ek] - k * NPC
        for r in range(R):
            seg[k, r] = (et[ek] == r).sum()
        deg = np.bincount(d_loc, minlength=NPC)
        csum = np.concatenate([[0], np.cumsum(deg)])
        n0 = cnt = 0
        while n0 < NPC:
            n1 = min(n0 + P, NPC)
            while csum[n1] - csum[n0] > WEDGE:
                n1 -= 1
            cnt += 1
            n0 = n1
        wcnt[k] = cnt
    S_r = ((seg.max(0) + CH - 1) // CH) * CH
    W = int(wcnt.max())

    Rm = np.asarray(inputs["rel_matrices"], np.float32)[0]  # [2, R, 2D, D]
    att = [np.asarray(inputs["att0"], np.float32).reshape(H, C),
           np.asarray(inputs["att1"], np.float32).reshape(H, C)]
    attblk = []
    for l in range(2):
        ab = np.zeros((D, H), np.float32)
        for h in range(H):
            ab[h * C:(h + 1) * C, h] = att[l][h]
        attblk.append(ab.astype(_bf))
    e4 = np.zeros((H, D), np.float32)
    for h in range(H):
        e4[h, h * C:(h + 1) * C] = 1.0
    common = {
        "embs_bf": np.asarray(inputs["embs"], np.float32).astype(_bf),
        "e4": e4.astype(_bf),
        "outw_bf": np.asarray(inputs["out_w"], np.float32).astype(_bf),
        "outb_bc": _bcast_row(inputs["out_b"]),
        "lng_bc": _bcast_row(inputs["ln_g"]),
        "lnb_bc": _bcast_row(inputs["ln_b"]),
    }
    for l, (wl, bl, we, bias) in enumerate([
            ("lin_l_w0", "lin_l_b0", "lin_edge_w0", "bias0"),
            ("lin_l_w1", "lin_l_b1", "lin_edge_w1", "bias1")]):
        for r in range(R):
            common[f"wRl_{l}_{r}"] = np.ascontiguousarray(Rm[l, r, :D, :]).astype(_bf)
            common[f"wRr_{l}_{r}"] = np.ascontiguousarray(Rm[l, r, D:, :]).astype(_bf)
        common[f"wWe_{l}"] = np.asarray(inputs[we], np.float32).astype(_bf)
        common[f"wWl_{l}"] = np.asarray(inputs[wl], np.float32).astype(_bf)
        common[f"attblk_{l}"] = attblk[l]
        common[f"bias_bc_{l}"] = _bcast_row(inputs[bias])
        common[f"blw_bc_{l}"] = _bcast_row(inputs[bl])

    in_maps = []
    for k in range(NC):
        m = dict(common)
        m.update(_prep_core(k, src, dst, et, S_r, W))
        in_maps.append(m)
    return S_r, W, in_maps


_CACHE = {}


def kernel(**inputs):
    S_r, W, in_maps = _host_prep(inputs)
    key = (tuple(S_r.tolist()), W)
    if key not in _CACHE:
        _CACHE[key] = build_nc(S_r, W)
    nc = _CACHE[key]
    res = run_bass_kernel_spmd(nc, in_maps, core_ids=list(range(NC)))
    return np.asarray(res.results[0]["y_out"], np.float32)


if __name__ == "__main__":
    print("import kernel; use test.py")
